# revision 7
# baseline (speedup 1.0000x reference)
"""Trainium2 Bass kernel for additive-attention nn.Module.

Math: reference computes
    scores[b,i,j] = x[b,i,:]@W[0,:3] + key[b,j,:]@W[0,3:] + b0
    attn = softmax(scores, axis=j) ; out = attn @ value

softmax over j is shift-invariant, so the x- and bias-terms (constant in j)
cancel exactly: attn[b,i,j] = softmax_j(key[b,j,:]@W[0,3:]) independent of i.
Hence out[b,i,:] = sum_j p[b,j] * value[b,j,:]  (identical for every i).

Kernel (data-parallel over batch, 8 batches/core on 8 cores). The per-core
work is a pure HBM-stream: read 2 MB of fp8 value, weighted-reduce over j.

v2 layout of this kernel (vs the 31.4us baseline):
  - value DMAs are the FIRST instructions in the program, split across the
    two HWDGE rings (sync: batches for arrivals 0,2,4,6; scalar: kil + the
    rest).  No SWDGE/gpsimd DMAs at all (Q7 descriptor-gen latency, ring
    memset preamble, and engine-7/15 contention all avoided).
  - per (batch, jj-chunk) the reduction is an M=1 matmul
      psum[1,256] += e_il[:, jj*8+b] (128x1 bf16) x v_chunk (128x256 fp8)
    with the 8 batches spread over the four PE column groups
    (tile_position=(0,32g)) so up to 4 chase streams run concurrently.
  - 1/s is broadcast to all 128 partitions with a tiny ones-matmul
    (B8[q,b] = 1/s[b]) so each batch's single psum row is normalized
    in place at its own partition 32g, then shipped as a 1 KB DMA.
  - a dummy Exp preloads the scalar-engine activation table during the
    DMA window; 4 small warm-up matmuls kick the PE HAM counter.
  - device output is just the unique rows: out_d[8, 256] f32 (8 KB).
    The S1=1024 broadcast happens during host-side unshard.
"""

import numpy as np
import ml_dtypes
from contextlib import ExitStack

import concourse.bass as bass
import concourse.bacc as bacc
import concourse.mybir as mybir
from concourse import tile
from concourse.bass_utils import run_bass_kernel_spmd

B, S1, S2, DV = 64, 1024, 1024, 256
NCORES = 8
BPC = B // NCORES            # batches per core
NJ = S2 // 128               # j-chunks / row-interleave factor
F32 = mybir.dt.float32
BF16 = mybir.dt.bfloat16
FP8 = mybir.dt.float8e3
FP8_NP = ml_dtypes.float8_e3m4

# processing order: (sync-ring batch, scalar-ring batch) pairs; arrival a
# uses column group a%4 and writes out_d row a.
ORDER = [0, 4, 1, 5, 2, 6, 3, 7]
N_WARM = 4

_compiled = {}


def _build_nc():
    nc = bacc.Bacc("TRN2", target_bir_lowering=False, debug=False,
                   num_devices=NCORES)

    kil_d = nc.dram_tensor("kil", [128, BPC * NJ * 3 + 3], F32,
                           kind="ExternalInput")
    val_d = nc.dram_tensor("value", [BPC, S2, DV], FP8, kind="ExternalInput")
    out_d = nc.dram_tensor("out", [BPC, DV], F32, kind="ExternalOutput")

    with tile.TileContext(nc) as tc, ExitStack() as ctx:
        sm = ctx.enter_context(tc.tile_pool(name="sm", bufs=1))
        vpool = ctx.enter_context(tc.tile_pool(name="v", bufs=BPC))
        ps_warm = ctx.enter_context(
            tc.tile_pool(name="ps_warm", bufs=1, space=bass.MemorySpace.PSUM))
        ps_s = ctx.enter_context(
            tc.tile_pool(name="ps_s", bufs=1, space=bass.MemorySpace.PSUM))
        ps_b8 = ctx.enter_context(
            tc.tile_pool(name="ps_b8", bufs=1, space=bass.MemorySpace.PSUM))
        ps_v = ctx.enter_context(
            tc.tile_pool(name="ps_v", bufs=1, space=bass.MemorySpace.PSUM))

        # ---- DMAs first: value stream starts at time zero ----
        kil_sb = sm.tile([128, BPC * NJ * 3 + 3], F32)
        nc.scalar.dma_start(kil_sb[:], kil_d[:])

        v_tiles = [None] * BPC
        for p in range(BPC // 2):
            bs, bc = ORDER[2 * p], ORDER[2 * p + 1]
            for b, eng in ((bs, nc.sync), (bc, nc.scalar)):
                v_sb = vpool.tile([128, NJ * DV], FP8, tag="v_sb")
                v_src = val_d.ap()[b].rearrange("(q jj) d -> q (jj d)", q=128)
                eng.dma_start(v_sb[:], v_src[:])
                v_tiles[b] = v_sb

        # ---- small consts; dummy Exp preloads the ACT table early ----
        dmy = sm.tile([1, 4], F32)
        nc.vector.memset(dmy[:], 0.0)
        dmy2 = sm.tile([1, 4], F32)
        nc.scalar.activation(dmy2[:], dmy[:],
                             mybir.ActivationFunctionType.Exp,
                             bias=0.0, scale=1.0)
        warm = sm.tile([128, 256], BF16)
        nc.vector.memset(warm[:], 0.0)
        ones_sb = sm.tile([128, BPC], BF16)
        nc.vector.memset(ones_sb[:], 1.0)
        ones8 = sm.tile([BPC, 128], F32)
        nc.vector.memset(ones8[:], 1.0 / BPC)

        # ---- PE warm-up (dependency-free, fills HAM activity window) ----
        wps = ps_warm.tile([BPC, 256], F32)
        for _ in range(N_WARM):
            nc.tensor.matmul(wps[:], warm[:, 0:BPC], warm[:],
                             start=True, stop=True)

        # ---- e_il[q, jj*8+b] = exp(key[b, 8q+jj, :] . w_k)  (bf16) ----
        wk_sb = kil_sb[:, BPC * NJ * 3:BPC * NJ * 3 + 3]
        k3 = kil_sb[:, 0:BPC * NJ * 3].rearrange("q (m f) -> q m f", f=3)
        t0 = sm.tile([128, BPC * NJ], F32)
        t1 = sm.tile([128, BPC * NJ], F32)
        t2 = sm.tile([128, BPC * NJ], F32)
        nc.vector.tensor_scalar_mul(t0[:], k3[:, :, 0], wk_sb[:, 0:1])
        nc.vector.scalar_tensor_tensor(
            t1[:], k3[:, :, 1], wk_sb[:, 1:2], t0[:],
            op0=mybir.AluOpType.mult, op1=mybir.AluOpType.add)
        nc.vector.scalar_tensor_tensor(
            t2[:], k3[:, :, 2], wk_sb[:, 2:3], t1[:],
            op0=mybir.AluOpType.mult, op1=mybir.AluOpType.add)
        e_il = sm.tile([128, BPC * NJ], BF16)
        nc.scalar.activation(e_il[:], t2[:], mybir.ActivationFunctionType.Exp,
                             bias=0.0, scale=1.0)

        # ---- s[b] = sum_j e ; rr[p, b] = 1/s[b] on partitions 0..7 ----
        s_ps = ps_s.tile([BPC, BPC * NJ], F32)
        nc.tensor.matmul(s_ps[:], ones_sb[:], e_il[:], start=True, stop=True)
        s8 = sm.tile([BPC, BPC], F32)
        nc.vector.tensor_reduce(
            s8[:], s_ps[:].rearrange("p (jj b) -> p b jj", b=BPC),
            axis=mybir.AxisListType.X, op=mybir.AluOpType.add)
        rr = sm.tile([BPC, BPC], F32)
        nc.vector.reciprocal(rr[:], s8[:])

        # ---- value reduction: chase the DMA stream, 4 PE column groups ----
        o_sb = sm.tile([128, BPC * DV], F32)
        b8_sb = sm.tile([128, BPC], F32)
        # one PSUM bank holds all 8 accumulator rows: arrival a lives at
        # partition 32*(a%4), columns (a//4)*DV
        ps_all = ps_v.tile([128, 2 * DV], F32)
        def ps_row(a):
            g = 32 * (a % 4)
            c = (a // 4) * DV
            return ps_all[g:g + 1, c:c + DV]
        for p in range(BPC // 2):
            a0, a1 = 2 * p, 2 * p + 1
            bs, bc = ORDER[a0], ORDER[a1]
            g0, g1 = 32 * (a0 % 4), 32 * (a1 % 4)
            for jj in range(NJ):
                nc.tensor.matmul(
                    ps_row(a0),
                    e_il[:, jj * BPC + bs:jj * BPC + bs + 1],
                    v_tiles[bs][:, jj * DV:(jj + 1) * DV],
                    start=(jj == 0), stop=(jj == NJ - 1),
                    tile_position=(0, g0))
                nc.tensor.matmul(
                    ps_row(a1),
                    e_il[:, jj * BPC + bc:jj * BPC + bc + 1],
                    v_tiles[bc][:, jj * DV:(jj + 1) * DV],
                    start=(jj == 0), stop=(jj == NJ - 1),
                    tile_position=(0, g1))
            if p == 0:
                # broadcast 1/s down all partitions: B8[q, b] = 1/s[b]
                b8_ps = ps_b8.tile([128, BPC], F32)
                nc.tensor.matmul(b8_ps[:], ones8[:], rr[:],
                                 start=True, stop=True)
                nc.vector.tensor_copy(b8_sb[:], b8_ps[:])
            # normalize each batch's single psum row in place, ship 1 KB
            nc.scalar.mul(o_sb[g0:g0 + 1, a0 * DV:(a0 + 1) * DV],
                          ps_row(a0),
                          b8_sb[g0:g0 + 1, bs:bs + 1])
            nc.sync.dma_start(out_d[a0:a0 + 1, :],
                              o_sb[g0:g0 + 1, a0 * DV:(a0 + 1) * DV])
            nc.vector.tensor_scalar_mul(
                o_sb[g1:g1 + 1, a1 * DV:(a1 + 1) * DV],
                ps_row(a1),
                b8_sb[g1:g1 + 1, bc:bc + 1])
            nc.sync.dma_start(out_d[a1:a1 + 1, :],
                              o_sb[g1:g1 + 1, a1 * DV:(a1 + 1) * DV])

    nc.compile()
    return nc


def _get_nc():
    if "nc" not in _compiled:
        _compiled["nc"] = _build_nc()
    return _compiled["nc"]


def _make_in_maps(key, value, W):
    key = np.asarray(key, dtype=np.float32)
    value = np.asarray(value, dtype=np.float32)
    W = np.asarray(W, dtype=np.float32)
    vq = value.astype(FP8_NP)
    wk128 = np.ascontiguousarray(np.tile(W[0, 3:].reshape(1, 3), (128, 1)))
    in_maps = []
    for c in range(NCORES):
        lo, hi = c * BPC, (c + 1) * BPC
        kc = key[lo:hi]                        # (BPC, S2, 3)
        # kil[q, (jj*BPC+b)*3+f] = key[b, interleaved row 8q+jj, f]
        kil = kc.reshape(BPC, 128, NJ, 3).transpose(1, 2, 0, 3)
        kil = kil.reshape(128, BPC * NJ * 3)
        kil = np.ascontiguousarray(np.concatenate([kil, wk128], axis=1))
        in_maps.append({
            "kil": kil,
            "value": np.ascontiguousarray(vq[lo:hi]),
        })
    return in_maps


def _finish(res):
    # device returns out[a] = normalized row of batch ORDER[a]
    parts = []
    for r in res.results:
        o = r["out"].reshape(BPC, DV)
        o8c = np.empty((BPC, DV), dtype=np.float32)
        for a, b in enumerate(ORDER):
            o8c[b] = o[a]
        parts.append(o8c)
    o8 = np.concatenate(parts, axis=0)         # (B, DV)
    full = np.broadcast_to(o8[:, None, :], (B, S1, DV))
    return np.ascontiguousarray(full)


def kernel(x, key, value, W, b):
    nc = _get_nc()
    in_maps = _make_in_maps(key, value, W)
    res = run_bass_kernel_spmd(nc, in_maps, core_ids=list(range(NCORES)))
    return _finish(res)


def kernel_traced(x, key, value, W, b, **spmd_kwargs):
    """Like kernel() but returns (output, BassKernelResults) — for test.py."""
    nc = _get_nc()
    in_maps = _make_in_maps(key, value, W)
    res = run_bass_kernel_spmd(nc, in_maps, core_ids=list(range(NCORES)),
                               **spmd_kwargs)
    return _finish(res), res


# revision 9
# speedup vs baseline: 1.0898x; 1.0898x over previous
"""Trainium2 Bass kernel for additive-attention nn.Module.

Math: reference computes
    scores[b,i,j] = x[b,i,:]@W[0,:3] + key[b,j,:]@W[0,3:] + b0
    attn = softmax(scores, axis=j) ; out = attn @ value

softmax over j is shift-invariant, so the x- and bias-terms (constant in j)
cancel exactly: attn[b,i,j] = softmax_j(key[b,j,:]@W[0,3:]) independent of i.
Hence out[b,i,:] = sum_j p[b,j] * value[b,j,:]  (identical for every i).

Kernel (data-parallel over batch, 8 batches/core on 8 cores). The per-core
work is a pure HBM-stream: read 2 MB of fp8 value, weighted-reduce over j.

v2 layout of this kernel (vs the 31.4us baseline):
  - value DMAs are the FIRST instructions in the program, split across the
    two HWDGE rings (sync: batches for arrivals 0,2,4,6; scalar: kil + the
    rest).  No SWDGE/gpsimd DMAs at all (Q7 descriptor-gen latency, ring
    memset preamble, and engine-7/15 contention all avoided).
  - per (batch, jj-chunk) the reduction is an M=1 matmul
      psum[1,256] += e_il[:, jj*8+b] (128x1 bf16) x v_chunk (128x256 fp8)
    with the 8 batches spread over the four PE column groups
    (tile_position=(0,32g)) so up to 4 chase streams run concurrently.
  - 1/s is broadcast to all 128 partitions with a tiny ones-matmul
    (B8[q,b] = 1/s[b]) so each batch's single psum row is normalized
    in place at its own partition 32g, then shipped as a 1 KB DMA.
  - a dummy Exp preloads the scalar-engine activation table during the
    DMA window; 4 small warm-up matmuls kick the PE HAM counter.
  - device output is just the unique rows: out_d[8, 256] f32 (8 KB).
    The S1=1024 broadcast happens during host-side unshard.
"""

import numpy as np
import ml_dtypes
from contextlib import ExitStack

import concourse.bass as bass
import concourse.bacc as bacc
import concourse.mybir as mybir
from concourse import tile
from concourse.bass_utils import run_bass_kernel_spmd

B, S1, S2, DV = 64, 1024, 1024, 256
NCORES = 8
BPC = B // NCORES            # batches per core
NJ = S2 // 128               # j-chunks / row-interleave factor
F32 = mybir.dt.float32
BF16 = mybir.dt.bfloat16
FP8 = mybir.dt.float8e3
FP8_NP = ml_dtypes.float8_e3m4

# value DMAs ride three rings (sync/scalar HWDGE + gpsimd SWDGE) because a
# single ring's descriptor generator only sustains ~130 GB/s with 2 KB lines.
# ARRIVAL[a] = device batch slot expected to land a-th; arrival a uses PE
# column group a%4, psum pair-tile a//2, and writes out_d row a.
SYNC_B = [0, 1, 2]
SCAL_B = [4, 5, 6]
GPS_B = [3, 7]
ARRIVAL = [0, 4, 3, 1, 5, 7, 2, 6]
N_WARM = 4

_compiled = {}


def _build_nc():
    nc = bacc.Bacc("TRN2", target_bir_lowering=False, debug=False,
                   num_devices=NCORES)

    kil_d = nc.dram_tensor("kil", [128, BPC * NJ * 3 + 3], F32,
                           kind="ExternalInput")
    val_d = nc.dram_tensor("value", [BPC, S2, DV], FP8, kind="ExternalInput")
    out_d = nc.dram_tensor("out", [BPC, DV], F32, kind="ExternalOutput")

    with tile.TileContext(nc) as tc, ExitStack() as ctx:
        sm = ctx.enter_context(tc.tile_pool(name="sm", bufs=1))
        vpool = ctx.enter_context(tc.tile_pool(name="v", bufs=BPC))
        ps_warm = ctx.enter_context(
            tc.tile_pool(name="ps_warm", bufs=1, space=bass.MemorySpace.PSUM))
        ps_s = ctx.enter_context(
            tc.tile_pool(name="ps_s", bufs=1, space=bass.MemorySpace.PSUM))
        ps_b8 = ctx.enter_context(
            tc.tile_pool(name="ps_b8", bufs=1, space=bass.MemorySpace.PSUM))
        ps_v = ctx.enter_context(
            tc.tile_pool(name="ps_v", bufs=4, space=bass.MemorySpace.PSUM))

        # ---- DMAs first: value stream starts at time zero ----
        kil_sb = sm.tile([128, BPC * NJ * 3 + 3], F32)
        nc.scalar.dma_start(kil_sb[:], kil_d[:])

        v_tiles = [None] * BPC
        # emit in per-ring FIFO order, interleaved across rings
        emit = []
        for i in range(3):
            if i < len(SYNC_B):
                emit.append((SYNC_B[i], nc.sync))
            if i < len(SCAL_B):
                emit.append((SCAL_B[i], nc.scalar))
            if i < len(GPS_B):
                emit.append((GPS_B[i], nc.gpsimd))
        for b, eng in emit:
            v_sb = vpool.tile([128, NJ * DV], FP8, tag="v_sb")
            v_src = val_d.ap()[b].rearrange("(q jj) d -> q (jj d)", q=128)
            eng.dma_start(v_sb[:], v_src[:])
            v_tiles[b] = v_sb

        # ---- small consts; dummy Exp preloads the ACT table early ----
        dmy = sm.tile([1, 4], F32)
        nc.vector.memset(dmy[:], 0.0)
        dmy2 = sm.tile([1, 4], F32)
        nc.scalar.activation(dmy2[:], dmy[:],
                             mybir.ActivationFunctionType.Exp,
                             bias=0.0, scale=1.0)
        warm = sm.tile([128, 256], BF16)
        nc.vector.memset(warm[:], 0.0)
        ones_sb = sm.tile([128, BPC], BF16)
        nc.vector.memset(ones_sb[:], 1.0)
        ones8 = sm.tile([BPC, 128], F32)
        nc.vector.memset(ones8[:], 1.0 / BPC)

        # ---- PE warm-up (dependency-free, fills HAM activity window) ----
        wps = ps_warm.tile([BPC, 256], F32)
        for _ in range(N_WARM):
            nc.tensor.matmul(wps[:], warm[:, 0:BPC], warm[:],
                             start=True, stop=True)

        # ---- e_il[q, jj*8+b] = exp(key[b, 8q+jj, :] . w_k)  (bf16) ----
        wk_sb = kil_sb[:, BPC * NJ * 3:BPC * NJ * 3 + 3]
        k3 = kil_sb[:, 0:BPC * NJ * 3].rearrange("q (m f) -> q m f", f=3)
        t0 = sm.tile([128, BPC * NJ], F32)
        t1 = sm.tile([128, BPC * NJ], F32)
        t2 = sm.tile([128, BPC * NJ], F32)
        nc.vector.tensor_scalar_mul(t0[:], k3[:, :, 0], wk_sb[:, 0:1])
        nc.vector.scalar_tensor_tensor(
            t1[:], k3[:, :, 1], wk_sb[:, 1:2], t0[:],
            op0=mybir.AluOpType.mult, op1=mybir.AluOpType.add)
        nc.vector.scalar_tensor_tensor(
            t2[:], k3[:, :, 2], wk_sb[:, 2:3], t1[:],
            op0=mybir.AluOpType.mult, op1=mybir.AluOpType.add)
        e_il = sm.tile([128, BPC * NJ], BF16)
        nc.scalar.activation(e_il[:], t2[:], mybir.ActivationFunctionType.Exp,
                             bias=0.0, scale=1.0)

        # ---- s[b] = sum_j e ; rr[p, b] = 1/s[b] on partitions 0..7 ----
        s_ps = ps_s.tile([BPC, BPC * NJ], F32)
        nc.tensor.matmul(s_ps[:], ones_sb[:], e_il[:], start=True, stop=True)
        s8 = sm.tile([BPC, BPC], F32)
        nc.vector.tensor_reduce(
            s8[:], s_ps[:].rearrange("p (jj b) -> p b jj", b=BPC),
            axis=mybir.AxisListType.X, op=mybir.AluOpType.add)
        rr = sm.tile([BPC, BPC], F32)
        nc.vector.reciprocal(rr[:], s8[:])

        # ---- value reduction: chase the DMA stream, 4 PE column groups ----
        o_sb = sm.tile([128, BPC * DV], F32)
        b8_sb = sm.tile([128, BPC], F32)
        # one psum tile per arrival-pair so pair p+1's matmuls carry no
        # false dependency on pair p's normalize read
        pair_ps = []
        for _p in range(BPC // 2):
            ppt = ps_v.tile([128, DV], F32, tag="pair_ps")
            pair_ps.append(ppt)
        for p in range(BPC // 2):
            a0, a1 = 2 * p, 2 * p + 1
            bs, bc = ARRIVAL[a0], ARRIVAL[a1]
            g0, g1 = 32 * (a0 % 4), 32 * (a1 % 4)
            for jj in range(NJ):
                nc.tensor.matmul(
                    pair_ps[p][g0:g0 + 1, :],
                    e_il[:, jj * BPC + bs:jj * BPC + bs + 1],
                    v_tiles[bs][:, jj * DV:(jj + 1) * DV],
                    start=(jj == 0), stop=(jj == NJ - 1),
                    tile_position=(0, g0))
                nc.tensor.matmul(
                    pair_ps[p][g1:g1 + 1, :],
                    e_il[:, jj * BPC + bc:jj * BPC + bc + 1],
                    v_tiles[bc][:, jj * DV:(jj + 1) * DV],
                    start=(jj == 0), stop=(jj == NJ - 1),
                    tile_position=(0, g1))
            if p == 0:
                # broadcast 1/s down all partitions: B8[q, b] = 1/s[b]
                b8_ps = ps_b8.tile([128, BPC], F32)
                nc.tensor.matmul(b8_ps[:], ones8[:], rr[:],
                                 start=True, stop=True)
                nc.vector.tensor_copy(b8_sb[:], b8_ps[:])
            # normalize each batch's single psum row in place, ship 1 KB
            nc.scalar.mul(o_sb[g0:g0 + 1, a0 * DV:(a0 + 1) * DV],
                          pair_ps[p][g0:g0 + 1, :],
                          b8_sb[g0:g0 + 1, bs:bs + 1])
            nc.sync.dma_start(out_d[a0:a0 + 1, :],
                              o_sb[g0:g0 + 1, a0 * DV:(a0 + 1) * DV])
            nc.vector.tensor_scalar_mul(
                o_sb[g1:g1 + 1, a1 * DV:(a1 + 1) * DV],
                pair_ps[p][g1:g1 + 1, :],
                b8_sb[g1:g1 + 1, bc:bc + 1])
            nc.sync.dma_start(out_d[a1:a1 + 1, :],
                              o_sb[g1:g1 + 1, a1 * DV:(a1 + 1) * DV])

    nc.compile()
    return nc


def _get_nc():
    if "nc" not in _compiled:
        _compiled["nc"] = _build_nc()
    return _compiled["nc"]


def _make_in_maps(key, value, W):
    key = np.asarray(key, dtype=np.float32)
    value = np.asarray(value, dtype=np.float32)
    W = np.asarray(W, dtype=np.float32)
    vq = value.astype(FP8_NP)
    wk128 = np.ascontiguousarray(np.tile(W[0, 3:].reshape(1, 3), (128, 1)))
    in_maps = []
    for c in range(NCORES):
        lo, hi = c * BPC, (c + 1) * BPC
        kc = key[lo:hi]                        # (BPC, S2, 3)
        # kil[q, (jj*BPC+b)*3+f] = key[b, interleaved row 8q+jj, f]
        kil = kc.reshape(BPC, 128, NJ, 3).transpose(1, 2, 0, 3)
        kil = kil.reshape(128, BPC * NJ * 3)
        kil = np.ascontiguousarray(np.concatenate([kil, wk128], axis=1))
        in_maps.append({
            "kil": kil,
            "value": np.ascontiguousarray(vq[lo:hi]),
        })
    return in_maps


def _finish(res):
    # device returns out[a] = normalized row of batch ARRIVAL[a]
    parts = []
    for r in res.results:
        o = r["out"].reshape(BPC, DV)
        o8c = np.empty((BPC, DV), dtype=np.float32)
        for a, b in enumerate(ARRIVAL):
            o8c[b] = o[a]
        parts.append(o8c)
    o8 = np.concatenate(parts, axis=0)         # (B, DV)
    full = np.broadcast_to(o8[:, None, :], (B, S1, DV))
    return np.ascontiguousarray(full)


def kernel(x, key, value, W, b):
    nc = _get_nc()
    in_maps = _make_in_maps(key, value, W)
    res = run_bass_kernel_spmd(nc, in_maps, core_ids=list(range(NCORES)))
    return _finish(res)


def kernel_traced(x, key, value, W, b, **spmd_kwargs):
    """Like kernel() but returns (output, BassKernelResults) — for test.py."""
    nc = _get_nc()
    in_maps = _make_in_maps(key, value, W)
    res = run_bass_kernel_spmd(nc, in_maps, core_ids=list(range(NCORES)),
                               **spmd_kwargs)
    return _finish(res), res


# revision 12
# speedup vs baseline: 1.1097x; 1.0182x over previous
"""Trainium2 Bass kernel for additive-attention nn.Module.

Math: reference computes
    scores[b,i,j] = x[b,i,:]@W[0,:3] + key[b,j,:]@W[0,3:] + b0
    attn = softmax(scores, axis=j) ; out = attn @ value

softmax over j is shift-invariant, so the x- and bias-terms (constant in j)
cancel exactly: attn[b,i,j] = softmax_j(key[b,j,:]@W[0,3:]) independent of i.
Hence out[b,i,:] = sum_j p[b,j] * value[b,j,:]  (identical for every i).

Kernel (data-parallel over batch, 8 batches/core on 8 cores). The per-core
work is a pure HBM stream: read 2 MB of fp8 value, weighted-reduce over j.

v4 structure (drives the stream at ~full HBM rate):
  - a dma_start blocks its issuing sequencer for descriptor-generation time
    (~16 ns per 2 KB descriptor on the HWDGE rings), so one ring tops out
    near ~125 GB/s.  Every batch's 256 KB is therefore split across all
    three rings (sync partitions 0:44, scalar 44:88, gpsimd 88:128) and the
    sub-DMAs are emitted batch-major: batches complete in order, evenly,
    at an aggregate ~300+ GB/s, and the matmul chase order is deterministic.
  - the e-chain exp sits between scalar-ring descriptor gens (after batch
    3's) so it issues right when its input is ready without stalling the
    value stream; a dummy Exp early on preloads the ACT table.
  - kil rides the gpsimd/SWDGE ring first (Q7 dispatch ~0.7us, cheap for
    the sequencer).
  - per (batch, jj-chunk) the reduction is an M=1 matmul
      psum[1,256] += e_il[:, jj*8+b] (128x1 bf16) x v_chunk (128x256 fp8)
    with column group b%4 (tile_position), emitted round-robin over batch
    quads so adjacent matmuls run concurrently on different column groups.
  - all 8 normalizes run on the vector ring (no DMAs there), reading the
    per-pair PSUM rows, scaled by B8[q,b] = 1/s[b] (a tiny ones-matmul
    broadcast); output ships as two 4 KB partition-strided DMAs.
  - device output is just the unique rows: out_d[4, 512] f32 (8 KB).
    The S1=1024 broadcast happens during host-side unshard.
"""

import numpy as np
import ml_dtypes
from contextlib import ExitStack

import concourse.bass as bass
import concourse.bacc as bacc
import concourse.mybir as mybir
from concourse import tile
from concourse.bass_utils import run_bass_kernel_spmd

B, S1, S2, DV = 64, 1024, 1024, 256
NCORES = 8
BPC = B // NCORES            # batches per core
NJ = S2 // 128               # j-chunks / row-interleave factor
F32 = mybir.dt.float32
BF16 = mybir.dt.bfloat16
FP8 = mybir.dt.float8e3
FP8_NP = ml_dtypes.float8_e3m4

# per-batch partition split across the three DMA rings
CUT0, CUT1 = 44, 88
N_WARM = 4

_compiled = {}


def _build_nc():
    nc = bacc.Bacc("TRN2", target_bir_lowering=False, debug=False,
                   num_devices=NCORES)

    kil_d = nc.dram_tensor("kil", [128, BPC * NJ * 3 + 3], F32,
                           kind="ExternalInput")
    val_d = nc.dram_tensor("value", [BPC, S2, DV], FP8, kind="ExternalInput")
    out_d = nc.dram_tensor("out", [4, 2 * DV], F32, kind="ExternalOutput")

    with tile.TileContext(nc) as tc, ExitStack() as ctx:
        sm = ctx.enter_context(tc.tile_pool(name="sm", bufs=1))
        vpool = ctx.enter_context(tc.tile_pool(name="v", bufs=BPC))
        ps_warm = ctx.enter_context(
            tc.tile_pool(name="ps_warm", bufs=1, space=bass.MemorySpace.PSUM))
        ps_s = ctx.enter_context(
            tc.tile_pool(name="ps_s", bufs=1, space=bass.MemorySpace.PSUM))
        ps_b8 = ctx.enter_context(
            tc.tile_pool(name="ps_b8", bufs=1, space=bass.MemorySpace.PSUM))
        ps_v = ctx.enter_context(
            tc.tile_pool(name="ps_v", bufs=4, space=bass.MemorySpace.PSUM))

        kil_sb = sm.tile([128, BPC * NJ * 3 + 3], F32)
        dmy = sm.tile([1, 4], F32)
        dmy2 = sm.tile([1, 4], F32)
        warm = sm.tile([128, 256], BF16)
        ones_sb = sm.tile([128, BPC], BF16)
        ones8 = sm.tile([BPC, 128], F32)
        t0 = sm.tile([128, BPC * NJ], F32)
        t1 = sm.tile([128, BPC * NJ], F32)
        t2 = sm.tile([128, BPC * NJ], F32)
        e_il = sm.tile([128, BPC * NJ], BF16)
        s8 = sm.tile([BPC, BPC], F32)
        rr = sm.tile([BPC, BPC], F32)
        b8_sb = sm.tile([128, BPC], F32)
        o_sb = sm.tile([128, 2 * DV], F32)

        # ---- value stream: every batch split across all three rings ----
        # kil first on gpsimd (cheap Q7 dispatch, done by ~1.5us)
        nc.gpsimd.dma_start(kil_sb[:], kil_d[:])
        v_tiles = []
        for b in range(BPC):
            v_sb = vpool.tile([128, NJ * DV], FP8, tag="v_sb")
            v_tiles.append(v_sb)
        for b in range(BPC):
            src = val_d.ap()[b].rearrange("(q jj) d -> q (jj d)", q=128)
            nc.sync.dma_start(v_tiles[b][0:CUT0, :], src[0:CUT0, :])
            nc.scalar.dma_start(v_tiles[b][CUT0:CUT1, :], src[CUT0:CUT1, :])
            nc.gpsimd.dma_start(v_tiles[b][CUT1:128, :], src[CUT1:128, :])
            if b == 0:
                # ACT-table preload issues after one scalar gen (~0.8us)
                nc.vector.memset(dmy[:], 0.0)
                nc.scalar.activation(dmy2[:], dmy[:],
                                     mybir.ActivationFunctionType.Exp,
                                     bias=0.0, scale=1.0)
                nc.vector.memset(warm[:], 0.0)
                nc.vector.memset(ones_sb[:], 1.0)
                nc.vector.memset(ones8[:], 1.0 / BPC)
            if b == 3:
                # e-chain: dots on vector, exp slotted into the scalar ring
                # here (t2 will be ready by the time it issues)
                wk_sb = kil_sb[:, BPC * NJ * 3:BPC * NJ * 3 + 3]
                k3 = kil_sb[:, 0:BPC * NJ * 3].rearrange(
                    "q (m f) -> q m f", f=3)
                nc.vector.tensor_scalar_mul(t0[:], k3[:, :, 0], wk_sb[:, 0:1])
                nc.vector.scalar_tensor_tensor(
                    t1[:], k3[:, :, 1], wk_sb[:, 1:2], t0[:],
                    op0=mybir.AluOpType.mult, op1=mybir.AluOpType.add)
                nc.vector.scalar_tensor_tensor(
                    t2[:], k3[:, :, 2], wk_sb[:, 2:3], t1[:],
                    op0=mybir.AluOpType.mult, op1=mybir.AluOpType.add)
                nc.scalar.activation(e_il[:], t2[:],
                                     mybir.ActivationFunctionType.Exp,
                                     bias=0.0, scale=1.0)

        # ---- PE warm-up (dependency-free, fills HAM activity window) ----
        wps = ps_warm.tile([BPC, 256], F32)
        for _ in range(N_WARM):
            nc.tensor.matmul(wps[:], warm[:, 0:BPC], warm[:],
                             start=True, stop=True)

        # ---- s[b] = sum_j e ; rr[p, b] = 1/s[b] on partitions 0..7 ----
        s_ps = ps_s.tile([BPC, BPC * NJ], F32)
        nc.tensor.matmul(s_ps[:], ones_sb[:], e_il[:], start=True, stop=True)
        nc.vector.tensor_reduce(
            s8[:], s_ps[:].rearrange("p (jj b) -> p b jj", b=BPC),
            axis=mybir.AxisListType.X, op=mybir.AluOpType.add)
        nc.vector.reciprocal(rr[:], s8[:])

        # ---- value reduction: chase batches in order, col group b%4 ----
        pair_ps = []
        for _p in range(BPC // 2):
            ppt = ps_v.tile([128, DV], F32, tag="pair_ps")
            pair_ps.append(ppt)

        def vmm(b, jj):
            g = 32 * (b % 4)
            nc.tensor.matmul(
                pair_ps[b // 2][g:g + 1, :],
                e_il[:, jj * BPC + b:jj * BPC + b + 1],
                v_tiles[b][:, jj * DV:(jj + 1) * DV],
                start=(jj == 0), stop=(jj == NJ - 1),
                tile_position=(0, g))

        for jj in range(NJ):
            for b in range(4):
                vmm(b, jj)
        # broadcast 1/s down all partitions: B8[q, b] = 1/s[b]
        b8_ps = ps_b8.tile([128, BPC], F32)
        nc.tensor.matmul(b8_ps[:], ones8[:], rr[:], start=True, stop=True)
        nc.vector.tensor_copy(b8_sb[:], b8_ps[:])
        for jj in range(NJ):
            for b in range(4, BPC):
                vmm(b, jj)

        # ---- normalize on the vector ring (no DMAs there) + ship ----
        o_v = o_sb[:].rearrange("(g r) c -> g r c", g=4)
        for b in range(BPC):
            g = 32 * (b % 4)
            c = (b // 4) * DV
            nc.vector.tensor_scalar_mul(
                o_sb[g:g + 1, c:c + DV],
                pair_ps[b // 2][g:g + 1, :],
                b8_sb[g:g + 1, b:b + 1])
            if b == 3:
                nc.sync.dma_start(out_d[:, 0:DV], o_v[:, 0, 0:DV])
        nc.sync.dma_start(out_d[:, DV:2 * DV], o_v[:, 0, DV:2 * DV])

    nc.compile()
    return nc


def _get_nc():
    if "nc" not in _compiled:
        _compiled["nc"] = _build_nc()
    return _compiled["nc"]


def _make_in_maps(key, value, W):
    key = np.asarray(key, dtype=np.float32)
    value = np.asarray(value, dtype=np.float32)
    W = np.asarray(W, dtype=np.float32)
    vq = value.astype(FP8_NP)
    wk128 = np.ascontiguousarray(np.tile(W[0, 3:].reshape(1, 3), (128, 1)))
    in_maps = []
    for c in range(NCORES):
        lo, hi = c * BPC, (c + 1) * BPC
        kc = key[lo:hi]                        # (BPC, S2, 3)
        # kil[q, (jj*BPC+b)*3+f] = key[b, interleaved row 8q+jj, f]
        kil = kc.reshape(BPC, 128, NJ, 3).transpose(1, 2, 0, 3)
        kil = kil.reshape(128, BPC * NJ * 3)
        kil = np.ascontiguousarray(np.concatenate([kil, wk128], axis=1))
        in_maps.append({
            "kil": kil,
            "value": np.ascontiguousarray(vq[lo:hi]),
        })
    return in_maps


def _finish(res):
    # device returns out[b%4, (b//4)*DV : ...] = normalized row of batch b
    parts = []
    for r in res.results:
        o = r["out"].reshape(4, 2 * DV)
        o8c = np.empty((BPC, DV), dtype=np.float32)
        for b in range(BPC):
            o8c[b] = o[b % 4, (b // 4) * DV:(b // 4 + 1) * DV]
        parts.append(o8c)
    o8 = np.concatenate(parts, axis=0)         # (B, DV)
    full = np.broadcast_to(o8[:, None, :], (B, S1, DV))
    return np.ascontiguousarray(full)


def kernel(x, key, value, W, b):
    nc = _get_nc()
    in_maps = _make_in_maps(key, value, W)
    res = run_bass_kernel_spmd(nc, in_maps, core_ids=list(range(NCORES)))
    return _finish(res)


def kernel_traced(x, key, value, W, b, **spmd_kwargs):
    """Like kernel() but returns (output, BassKernelResults) — for test.py."""
    nc = _get_nc()
    in_maps = _make_in_maps(key, value, W)
    res = run_bass_kernel_spmd(nc, in_maps, core_ids=list(range(NCORES)),
                               **spmd_kwargs)
    return _finish(res), res


# revision 19
# speedup vs baseline: 1.2013x; 1.0826x over previous
"""Trainium2 Bass kernel for additive-attention nn.Module.

Math: reference computes
    scores[b,i,j] = x[b,i,:]@W[0,:3] + key[b,j,:]@W[0,3:] + b0
    attn = softmax(scores, axis=j) ; out = attn @ value

softmax over j is shift-invariant, so the x- and bias-terms (constant in j)
cancel exactly: attn[b,i,j] = softmax_j(key[b,j,:]@W[0,3:]) independent of i.
Hence out[b,i,:] = sum_j p[b,j] * value[b,j,:]  (identical for every i).

Kernel (data-parallel over batch, 8 batches/core on 8 cores). The per-core
work is a pure HBM stream: read 2 MB of fp8 value, weighted-reduce over j.

v5 structure. Measured facts driving it: a dma_start blocks its issuing
sequencer for a ~600-770 ns fixed DIRECT2D dispatch (descriptor count
barely matters), single-partition DVE ops cost ~0.5 us, and engine-to-
engine semaphore wakeups cost ~0.5-1 us.  So:
  - value moves as 8 whole-batch DMAs (256 KB each, 2 KB/partition
    descriptors): sync ring batches [0,1,2], scalar [3,4,5], gpsimd
    [kil,6,7].  Three rings drain concurrently at the HBM limit; batch
    completion order is ~[0,3,6,1,4,7,2,5] (chase order ARRIVAL).
  - per (batch, jj-chunk) the reduction is an M=1 matmul
      psum[1,256] += e_il[:, jj*8+b] (128x1 bf16) x v_chunk (128x256 fp8)
    at column group a%4 (a = arrival index), so 4 chase streams run
    concurrently on the PE; each arrival-quad accumulates into ONE psum
    tile at partitions {0,32,64,96}.
  - normalization: 1/s is routed to partition 32g once via
    PE-transpose(rr) -> mask -> block-indicator matmul, giving
    b8x[q,h] = 1/s[batch(g,h)]; each quad then normalizes with a single
    [4,256] partition-strided multiply and ships as one 4 KB DMA.
  - the e-chain exp issues on the scalar ring after its three value
    gens; a dummy Exp up front preloads the ACT table.
  - device output is out_d[4, 512] f32 (8 KB): row g col-half h = batch
    ARRIVAL[h*4+g].  The S1=1024 broadcast happens during host unshard.
"""

import numpy as np
import ml_dtypes
from contextlib import ExitStack

import concourse.bass as bass
import concourse.bacc as bacc
import concourse.mybir as mybir
from concourse import tile
from concourse.bass_utils import run_bass_kernel_spmd

B, S1, S2, DV = 64, 1024, 1024, 256
NCORES = 8
BPC = B // NCORES            # batches per core
NJ = S2 // 128               # j-chunks / row-interleave factor
F32 = mybir.dt.float32
BF16 = mybir.dt.bfloat16
FP8 = mybir.dt.float8e3
FP8_NP = ml_dtypes.float8_e3m4

SYNC_B = [0, 1, 2]
SCAL_B = [3, 4, 5]
GPS_B = [6, 7]
ARRIVAL = [0, 3, 6, 1, 4, 7, 2, 5]
N_WARM = 4

_compiled = {}


def _build_nc():
    nc = bacc.Bacc("TRN2", target_bir_lowering=False, debug=False,
                   num_devices=NCORES)

    kil_d = nc.dram_tensor("kil", [128, 195], F32, kind="ExternalInput")
    val_d = nc.dram_tensor("value", [BPC, S2, DV], FP8, kind="ExternalInput")
    out_d = nc.dram_tensor("out", [4, 2 * DV], F32, kind="ExternalOutput")

    with tile.TileContext(nc) as tc, ExitStack() as ctx:
        sm = ctx.enter_context(tc.tile_pool(name="sm", bufs=1))
        vpool = ctx.enter_context(tc.tile_pool(name="v", bufs=BPC))
        ps_warm = ctx.enter_context(
            tc.tile_pool(name="ps_warm", bufs=1, space=bass.MemorySpace.PSUM))
        ps_s = ctx.enter_context(
            tc.tile_pool(name="ps_s", bufs=1, space=bass.MemorySpace.PSUM))
        ps_b8 = ctx.enter_context(
            tc.tile_pool(name="ps_b8", bufs=1, space=bass.MemorySpace.PSUM))
        ps_v = ctx.enter_context(
            tc.tile_pool(name="ps_v", bufs=4, space=bass.MemorySpace.PSUM))

        kil_sb = sm.tile([128, 195], F32)
        dmy = sm.tile([1, 4], F32)
        dmy2 = sm.tile([1, 4], F32)
        warm = sm.tile([128, 256], BF16)
        ones_sb = sm.tile([128, BPC], BF16)
        ones8 = sm.tile([BPC, 128], F32)
        t0 = sm.tile([128, BPC * NJ], F32)
        t1 = sm.tile([128, BPC * NJ], F32)
        t2 = sm.tile([128, BPC * NJ], F32)
        e_il = sm.tile([128, BPC * NJ], BF16)
        s8 = sm.tile([BPC, BPC], F32)
        rr = sm.tile([BPC, BPC], F32)
        b8_sb = sm.tile([128, BPC], F32)
        o_sb = sm.tile([128, 2 * DV], F32)

        # ---- value stream: whole-batch DMAs, 3 rings ----
        nc.gpsimd.dma_start(kil_sb[:], kil_d[:])
        v_tiles = [None] * BPC
        for b in range(BPC):
            v_sb = vpool.tile([128, NJ * DV], FP8, tag="v_sb")
            v_tiles[b] = v_sb
        # dummy Exp first on scalar ring: ACT table preload (needs dmy)
        nc.vector.memset(dmy[:], 0.0)
        nc.scalar.activation(dmy2[:], dmy[:],
                             mybir.ActivationFunctionType.Exp,
                             bias=0.0, scale=1.0)
        for i in range(3):
            for blist, eng in ((SYNC_B, nc.sync), (SCAL_B, nc.scalar),
                               (GPS_B, nc.gpsimd)):
                if i < len(blist):
                    b = blist[i]
                    src = val_d.ap()[b].rearrange(
                        "(q jj) d -> q (jj d)", q=128)
                    eng.dma_start(v_tiles[b][:], src[:])

        nc.vector.memset(warm[:], 0.0)
        nc.vector.memset(ones_sb[:], 1.0)
        nc.vector.memset(ones8[:], 1.0 / BPC)

        # ---- PE warm-up (dependency-free, fills HAM activity window) ----
        wps = ps_warm.tile([BPC, 256], F32)
        for _ in range(N_WARM):
            nc.tensor.matmul(wps[:], warm[:, 0:BPC], warm[:],
                             start=True, stop=True)

        # ---- e_il[q, jj*8+b] = exp(key[b, 8q+jj, :] . w_k)  (bf16) ----
        wk_sb = kil_sb[:, 192:195]
        k3 = kil_sb[:, 0:192].rearrange("q (m f) -> q m f", f=3)
        nc.vector.tensor_scalar_mul(t0[:], k3[:, :, 0], wk_sb[:, 0:1])
        nc.vector.scalar_tensor_tensor(
            t1[:], k3[:, :, 1], wk_sb[:, 1:2], t0[:],
            op0=mybir.AluOpType.mult, op1=mybir.AluOpType.add)
        nc.vector.scalar_tensor_tensor(
            t2[:], k3[:, :, 2], wk_sb[:, 2:3], t1[:],
            op0=mybir.AluOpType.mult, op1=mybir.AluOpType.add)
        nc.scalar.activation(e_il[:], t2[:], mybir.ActivationFunctionType.Exp,
                             bias=0.0, scale=1.0)

        # ---- s[b] = sum_j e ; rr[p, b] = 1/s[b] on partitions 0..7 ----
        s_ps = ps_s.tile([BPC, BPC * NJ], F32)
        nc.tensor.matmul(s_ps[:], ones_sb[:], e_il[:], start=True, stop=True)
        nc.vector.tensor_reduce(
            s8[:], s_ps[:].rearrange("p (jj b) -> p b jj", b=BPC),
            axis=mybir.AxisListType.X, op=mybir.AluOpType.add)
        nc.vector.reciprocal(rr[:], s8[:])

        # ---- value reduction: chase batches, col group a%4 ----
        # one psum tile per arrival-pair (2 concurrent column-group
        # streams per PSUM bank; 4 corrupts)
        pair_ps = []
        for _p in range(4):
            ppt = ps_v.tile([128, DV], F32, tag="pair_ps")
            pair_ps.append(ppt)

        def vmm(a, jj):
            b = ARRIVAL[a]
            g = 32 * (a % 4)
            nc.tensor.matmul(
                pair_ps[a // 2][g:g + 1, :],
                e_il[:, jj * BPC + b:jj * BPC + b + 1],
                v_tiles[b][:, jj * DV:(jj + 1) * DV],
                start=(jj == 0), stop=(jj == NJ - 1),
                tile_position=(0, g))

        for a in range(4):
            for jj in range(NJ):
                vmm(a, jj)
        # broadcast 1/s down all partitions: B8[q, b] = 1/s[b]
        b8_ps = ps_b8.tile([128, BPC], F32)
        nc.tensor.matmul(b8_ps[:], ones8[:], rr[:], start=True, stop=True)
        nc.vector.tensor_copy(b8_sb[:], b8_ps[:])
        for a in range(4, BPC):
            for jj in range(NJ):
                vmm(a, jj)

        # ---- per-batch normalize spread over 3 engines + ship ----
        o_v = o_sb[:].rearrange("(g r) c -> g r c", g=4)
        norm_eng = [nc.vector, nc.scalar]
        for h in range(2):
            for i in range(4):
                a = h * 4 + i
                b = ARRIVAL[a]
                g = 32 * (a % 4)
                eng = norm_eng[a % 2]
                if eng is nc.scalar:
                    eng.mul(o_sb[g:g + 1, h * DV:(h + 1) * DV],
                            pair_ps[a // 2][g:g + 1, :],
                            b8_sb[g:g + 1, b:b + 1])
                else:
                    eng.tensor_scalar_mul(
                        o_sb[g:g + 1, h * DV:(h + 1) * DV],
                        pair_ps[a // 2][g:g + 1, :],
                        b8_sb[g:g + 1, b:b + 1])
            nc.sync.dma_start(out_d[:, h * DV:(h + 1) * DV],
                              o_v[:, 0, h * DV:(h + 1) * DV])

    nc.compile()
    return nc


def _get_nc():
    if "nc" not in _compiled:
        _compiled["nc"] = _build_nc()
    return _compiled["nc"]


def _make_in_maps(key, value, W):
    key = np.asarray(key, dtype=np.float32)
    value = np.asarray(value, dtype=np.float32)
    W = np.asarray(W, dtype=np.float32)
    vq = value.astype(FP8_NP)
    wk128 = np.tile(W[0, 3:].reshape(1, 3), (128, 1)).astype(np.float32)
    in_maps = []
    for c in range(NCORES):
        lo, hi = c * BPC, (c + 1) * BPC
        kc = key[lo:hi]                        # (BPC, S2, 3)
        # kil[q, (jj*BPC+b)*3+f] = key[b, interleaved row 8q+jj, f]
        kil = kc.reshape(BPC, 128, NJ, 3).transpose(1, 2, 0, 3)
        kil = kil.reshape(128, BPC * NJ * 3)
        kil = np.ascontiguousarray(np.concatenate([kil, wk128], axis=1))
        in_maps.append({
            "kil": kil,
            "value": np.ascontiguousarray(vq[lo:hi]),
        })
    return in_maps


def _finish(res):
    # device out[g, h*DV:...] = normalized row of batch ARRIVAL[h*4+g]
    parts = []
    for r in res.results:
        o = r["out"].reshape(4, 2 * DV)
        o8c = np.empty((BPC, DV), dtype=np.float32)
        for a in range(BPC):
            g, h = a % 4, a // 4
            o8c[ARRIVAL[a]] = o[g, h * DV:(h + 1) * DV]
        parts.append(o8c)
    o8 = np.concatenate(parts, axis=0)         # (B, DV)
    full = np.broadcast_to(o8[:, None, :], (B, S1, DV))
    return np.ascontiguousarray(full)


def kernel(x, key, value, W, b):
    nc = _get_nc()
    in_maps = _make_in_maps(key, value, W)
    res = run_bass_kernel_spmd(nc, in_maps, core_ids=list(range(NCORES)))
    return _finish(res)


def kernel_traced(x, key, value, W, b, **spmd_kwargs):
    """Like kernel() but returns (output, BassKernelResults) — for test.py."""
    nc = _get_nc()
    in_maps = _make_in_maps(key, value, W)
    res = run_bass_kernel_spmd(nc, in_maps, core_ids=list(range(NCORES)),
                               **spmd_kwargs)
    return _finish(res), res
